# revision 37
# baseline (speedup 1.0000x reference)
"""Trainium2 Bass kernel for a causal AttentionBlock (dense transformer).

Model (reference):
    qkv = x @ Wqkv + bqkv ; 16-head causal attention (no out-proj)
    x2  = x + attn_out
    out = x2 + relu(x2 @ W1 + b1) @ W2 + b2

x: [2, 2048, 1024] fp32. 8 NeuronCores.

Sharding (no collectives): data-parallel over (batch, query-chunk). Core c
handles batch b = c//4 and the balanced causal chunk pair (j, 7-j), j = c%4,
of 8x256-row chunks, giving every core the same 512 query rows. Each core
redundantly projects K/V for its whole batch (uniform SPMD program), computes
attention for its rows with shipped additive gates/mask, then the MLP.

Everything on-chip runs transposed ([feature, row] layout). The host ships
x pre-transposed (fp32 for the residual path, fp8/fp16 for matmul operands)
and transposes the output back, so the PE does no transposes at all.
Weights are host-packed so every DMA is contiguous full rows.

Precision: Q/K/V projections, the AV matmuls, and 24 of 32 MLP2 hidden
blocks run fp8(e4m3) in DoubleRow mode; the 8-block fp16 MLP2 tail (both
halves pre-scaled so they share one 2048x PSUM accumulation) caps the
quantization noise at ~1.67e-2 total rel err (full-fp8 MLP2 measured
1.92e-2 — too close to the 2e-2 gate). Weights are host-scaled by 64 into
e4m3's normal range; kth stores 64*(k+bk) and the Q evacuation folds
0.125/64^2 so scores come out exact. MLP1 stays fp16; its relu output is
written as fp8(32*h) / fp16(32*h) with b1 shipped pre-scaled by 32.

Attention processes BOTH heads of a feature block together, chunk by
chunk: the 4 score matmuls of a 256-kv block alternate PE row groups
(h0 at partitions 0:64, h1 at 64:128, auto tile_position) so they run
pairwise-concurrent, and one 1024-wide Exp covers (h, s, q) — the gate
bias is per (slot, chunk), identical for both heads. AV accumulates
[64v|1]-augmented values per head into free-dim halves of a per-chunk
accumulator bank; only the chunk's very first AV uses start=True because
start clears has_written for the WHOLE bank (see mk_avs). The finalize
(Ln, exp(-ln den), a fp16 ones*(1/64) broadcast matmul that borrows the
psP rotation — drain_ev first! — then normalize+residual) runs once per
chunk at width 512; h1's rows shift to partitions 64:128 via gpsimd
SBUF-SBUF DMA. rec = 1/den stays fp16-normal; the 1/64 that undoes
vaug2's 64*v lives in the broadcast operand (1/(64*den) would go fp16-
subnormal and the PE flushes subnormals to zero). K(f+1), V(u1..3), and
MLP1's first-half partials interleave into attention(f)'s stream to keep
the PE dense (HAM halves the clock after ~3.4us of idle). Input DMAs
spread across the sync/scalar/gpsimd queues; phase-3's first w1b tiles
prefetch during late attention into a pool outside the attention pools'
bytes.
"""
import os
import sys

sys.path.insert(0, "/opt/trn_rl_repo")

import numpy as np

import bass_rust
import concourse.bass as bass
import concourse.mybir as mybir
import concourse.tile as tile
from concourse.bass_utils import run_bass_kernel_spmd

# ---------------------------------------------------------------- constants
B, T, N = 2, 2048, 1024
H, D = 16, 64
NCORES = 8
CH = 256               # query chunk rows
F32 = mybir.dt.float32
F32R = mybir.dt.float32r
F16 = mybir.dt.float16
F8 = mybir.dt.float8e4
U16 = mybir.dt.uint16
U32 = mybir.dt.uint32

# fp8(e4m3) weights are host-scaled by 64 into e4m3's normal range; the
# consumers descale (see module docstring).
W8SCALE = 64.0

_prog_cache = {}


# ------------------------------------------------------------- wait legalizer
def _legalize_waits(nc):
    """This walrus build accepts <=1 sync wait on most instructions and 0 on
    4-byte-input Matmult (fused self-loading LDW). Move excess waits onto bare
    EventSemaphore instructions inserted before, on the same engine."""
    n_split = 0
    for fn in nc.m.functions:
        for blk in fn.blocks:
            insts = blk.instructions
            out = []
            for inst in insts:
                si = inst.sync_info
                waits = list(si.on_wait) if si is not None else []
                tname = type(inst).__name__
                if tname in ("InstMatmult", "InstMatmultMx"):
                    maxw = 0
                    for arg in inst.ins:
                        dt = getattr(arg, "dtype", None)
                        if dt is not None and mybir.dt.size(dt) == 2:
                            maxw = 1
                            break
                else:
                    maxw = 1
                if len(waits) > maxw:
                    extra = waits[:-maxw] if maxw else waits
                    keep = waits[-maxw:] if maxw else []
                    for k, w in enumerate(extra):
                        ev = mybir.InstEventSemaphore(
                            name=f"{inst.name}-lw{k}", ins=[], outs=[]
                        )
                        ev.engine = inst.engine
                        ev.sync_info = bass_rust.SyncInfo(on_wait=[w], on_update=[])
                        out.append(ev)
                        n_split += 1
                    inst.sync_info = bass_rust.SyncInfo(
                        on_wait=keep, on_update=list(si.on_update)
                    )
                out.append(inst)
            insts[:] = out
    return n_split


# ------------------------------------------------------------------- program
def _build_program():
    nc = bass.Bass("TRN2", debug=False, num_devices=NCORES)

    t_ = {}
    t_["xqt"] = nc.dram_tensor("xqt", [N, 2 * CH], F32, kind="ExternalInput").ap()
    # fp8(e4m3) DoubleRow operands: [kc2][p, i, cols] pairs feature blocks
    # (2*kc2, 2*kc2+1) along the free dim
    t_["xq8"] = nc.dram_tensor("xq8", [4, 128, 2, 2 * CH], F8,
                               kind="ExternalInput").ap()
    t_["xb8"] = nc.dram_tensor("xb8", [128, 4, 2, T], F8,
                               kind="ExternalInput").ap()
    t_["wq_t"] = nc.dram_tensor("wq_t", [128, 8, 4, 2, 128], F8,
                                kind="ExternalInput").ap()
    t_["wk_t"] = nc.dram_tensor("wk_t", [128, 8, 4, 2, 128], F8,
                                kind="ExternalInput").ap()
    t_["wv_t"] = nc.dram_tensor("wv_t", [4, 4, 128, 2, 256], F8,
                                kind="ExternalInput").ap()
    t_["w1a"] = nc.dram_tensor("w1a", [32, 128, 4, 128], F16,
                                kind="ExternalInput").ap()
    t_["w1b"] = nc.dram_tensor("w1b", [32, 128, 4, 128], F16,
                               kind="ExternalInput").ap()
    t_["w2_t"] = nc.dram_tensor("w2_t", [8, 128, 32, 128], F16,
                                kind="ExternalInput").ap()
    t_["ball"] = nc.dram_tensor("ball", [128, 64], F32,
                                kind="ExternalInput").ap()
    t_["gates"] = nc.dram_tensor("gates", [128, 16], F32,
                                 kind="ExternalInput").ap()
    t_["maskd"] = nc.dram_tensor("maskd", [256, CH], F32,
                                 kind="ExternalInput").ap()
    t_["out_t"] = nc.dram_tensor("out_t", [N, 2 * CH], F32,
                                 kind="ExternalOutput").ap()

    with tile.TileContext(nc) as tc:
        _emit(nc, tc, t_)
    return nc


def _emit(nc, tc, t_):
    AF = mybir.ActivationFunctionType
    OP = mybir.AluOpType

    with tc.tile_pool(name="const", bufs=1) as const:
        # broadcast operand holds 1/64 (fp16 0x2400), not 1.0: rec = 1/den
        # stays in fp16 normal range (1/(64*den) would go subnormal and the
        # PE flushes fp16 subnormals to zero); the 1/64 that undoes vaug2's
        # 64*v rides in here instead
        ones16 = const.tile([128, 64], F16)
        nc.vector.memset(ones16[:].bitcast(U16), 0x2400)
        # all five bias vectors ride in one [128, 64] tile / one DMA;
        # bias[nm] are AP views into it (5 separate small DMAs cost ~600ns
        # of descriptor overhead EACH and starved the Q evacuations)
        ball = const.tile([128, 64], F32)
        bias = {"bqs": ball[:, 0:8], "bk": ball[:, 8:16],
                "bv": ball[:, 16:24], "b1": ball[:, 24:56],
                "b2": ball[:, 56:64]}
        gt = const.tile([128, 16], F32)
        # diag mask in (head, s, q) layout so one Exp covers both heads
        md = const.tile([128, 2, 2, CH], F32)

        with tc.tile_pool(name="outer", bufs=1) as pout:
            x2t = [pout.tile([128, 2 * CH], F32, tag=f"x2t{f}", name=f"x2t{f}")
                   for f in range(8)]
            x2r = [pout.tile([128, 2 * CH], F16, tag=f"x2r{f}",
                             name=f"x2r{f}") for f in range(8)]
            # MLP1 first-half partial sums (kc 0..3), produced inside the
            # ACT-bound attention tail where the PE is otherwise idle
            p1 = [pout.tile([128, 2 * CH], F16, tag=f"p1_{m}",
                            name=f"p1_{m}") for m in range(32)]

            # w1b tiles for the first phase-3 chains live in their own pool
            # (outside the attention pools' bytes) so their DMAs can run
            # during late attention instead of waiting for kth's last read
            with tc.tile_pool(name="pre3", bufs=1) as pre3:
                pre3_w1 = {}
                with tc.tile_pool(name="keep", bufs=1) as keep, \
                     tc.tile_pool(name="pw", bufs=2) as pw, \
                     tc.tile_pool(name="p2w", bufs=2) as p2w, \
                     tc.tile_pool(name="psP", bufs=2, space="PSUM") as psP, \
                     tc.tile_pool(name="psS", bufs=2, space="PSUM") as psS, \
                     tc.tile_pool(name="psA", bufs=2, space="PSUM") as psA:
                    _phase12(nc, tc, AF, OP, t_, ball, gt, md, ones16,
                             keep, pw, p2w, psP, psS, psA, x2t, x2r, p1,
                             pre3, pre3_w1)
                _phase3(nc, tc, AF, OP, t_, ball, x2t, x2r, p1, pre3_w1)


def _phase12(nc, tc, AF, OP, t_, ball, gt, md, ones16,
             keep, pw, p2w, psP, psS, psA, x2t, x2r, p1, pre3, pre3_w1):
    """Q/K/V projections fused with attention; writes x2t/x2r."""
    # ---- persistent tiles
    xqt = [keep.tile([128, 2 * CH], F32, tag=f"xqt{f}", name=f"xqt{f}")
           for f in range(8)]
    xq8 = [keep.tile([128, 2, 2 * CH], F8, tag=f"xq8_{i}", name=f"xq8_{i}")
           for i in range(4)]
    xb8t = keep.tile([128, 4, 2, T], F8, tag="xb8t", name="xb8t")
    qt = [keep.tile([128, 2 * CH], F16, tag=f"qt{f}", name=f"qt{f}")
          for f in range(8)]
    # vaug2[blk]: kv-block pair (2*blk, 2*blk+1) along dim 1, matching the
    # ex tiles' s dimension for DoubleRow AV; [64*v | 1] per head
    vaug2 = [keep.tile([128, 2, H, D + 1], F8, tag=f"va{b2}", name=f"va{b2}")
             for b2 in range(T // 256)]

    # ---- input DMAs, spread over four queues so the prologue is not
    # serialized behind two ~200GB/s DMA rings: xq8 on scalar (feeds the
    # first Q matmuls), xb8 halves split vector/gpsimd (feed K/V), weights
    # stay on sync.
    nc.scalar.dma_start(ball[:], t_["ball"])
    for i in range(4):
        nc.scalar.dma_start(
            xq8[i][:].rearrange("p a b -> p (a b)"),
            t_["xq8"][i].rearrange("p a b -> p (a b)"))
    for cb, eng in ((0, nc.scalar), (1, nc.gpsimd)):
        eng.dma_start(
            xb8t[:, :, :, cb * 1024:(cb + 1) * 1024],
            t_["xb8"][:, :, :, cb * 1024:(cb + 1) * 1024])
    # gate/mask loads ride behind the critical xq8/xb8 halves on scalar
    nc.scalar.dma_start(gt[:], t_["gates"])
    for hp in range(2):
        nc.scalar.dma_start(
            md[:, hp, :, :], t_["maskd"].rearrange("(c p) q -> p c q", p=128))

    # xqt (residual operand) is not needed until the first finalize; keep it
    # off the sync queue so weight stages are not delayed behind it
    for f in range(8):
        nc.gpsimd.dma_start(xqt[f][:], t_["xqt"][f * 128:(f + 1) * 128, :])

    # vaug2 augmentation column: [64v|1] for every head (e4m3 1.0 = 0x38)
    for b2 in range(T // 256):
        nc.vector.memset(vaug2[b2][:, :, :, D:D + 1].bitcast(mybir.dt.uint8),
                         0x38)

    # fold the V-projection bias into the residual operand (softmax
    # weights sum to 1, so attn(v + bv) = attn(v) + bv)
    for f in range(8):
        nc.vector.tensor_scalar_add(xqt[f][:], xqt[f][:],
                                    ball[:, 16 + f:17 + f])

    # HAM warm-up: ~5.5us of dummy matmuls into a never-read PSUM scratch
    # slot while the input DMAs land. The clock gate defaults to 4/8
    # (1.2GHz) and needs ~3.4us of sustained PE activity to flip; 48 MMs
    # (2.8us) measured too short — the warm transition still waited for
    # the real stream. 96 MMs x ~58ns clears the window with margin.
    wu = psS.tile([128, 2, 2, CH], F32, tag="ps", name="wu")
    for _ in range(96):
        nc.tensor.matmul(wu[0:64, 0, 0, 0:64], ones16[:, :], ones16[:, :],
                         start=True, stop=True)

    ev_pend = [None]

    def defer_ev(fn):
        if ev_pend[0] is not None:
            ev_pend[0]()
        ev_pend[0] = fn

    def drain_ev():
        if ev_pend[0] is not None:
            ev_pend[0]()
        ev_pend[0] = None

    DR = mybir.MatmulPerfMode.DoubleRow

    # ---- Q projection (fp8 DoubleRow): qt[f] = ((64Wq).T@xq)*QS + bq*0.125/64
    # qt carries an extra 1/64 so scores against kth's 64x come out exact.
    QS = 0.125 / (64.0 * 64.0)
    wqs = {}

    def load_wq(f):
        wqs[f] = pw.tile([128, 4, 2, 128], F8, tag="wqk", bufs=4,
                         name=f"wq{f}")
        nc.sync.dma_start(
            wqs[f][:].rearrange("p a b c -> p (a b c)"),
            t_["wq_t"][f].rearrange("p a b c -> p (a b c)"))

    load_wq(0)
    load_wq(1)
    for f in range(8):
        if f + 2 < 8:
            load_wq(f + 2)
        wq = wqs.pop(f)
        pp = psP.tile([128, 2 * CH], F32, tag="proj")
        for kc2 in range(4):
            nc.tensor.matmul(pp[:], wq[:, kc2, :, :], xq8[kc2][:],
                             start=(kc2 == 0), stop=(kc2 == 3), perf_mode=DR)
        defer_ev(lambda pp=pp, f=f: nc.vector.tensor_scalar(
            out=qt[f][:], in0=pp[:], scalar1=QS,
            scalar2=ball[:, 0 + f:1 + f], op0=OP.mult, op1=OP.add))

    # ---- V projection for one 256-wide column unit (4 heads: 4u..4u+3).
    # Unit u is first consumed by attention step f = 2u, so later units can
    # interleave as background PE work deep into the attention phase.
    # vaug keeps 64*v; the finalize folds the 1/64 into exp(-ln(den)-ln64).
    def emit_vproj_groups(u):
        """Returns a list of thunks; each emits one rt-group (4 matmuls)."""
        wvs = [pw.tile([128, 2, 256], F8, tag="wv", bufs=8,
                       name=f"wv{u}_{i}") for i in range(4)]
        for kc2 in range(4):
            nc.sync.dma_start(
                wvs[kc2][:].rearrange("p a b -> p (a b)"),
                t_["wv_t"][u, kc2].rearrange("p a b -> p (a b)"))

        def mk(rt):
            def go():
                pp = psP.tile([128, 512], F32, tag="proj")
                for kc2 in range(4):
                    nc.tensor.matmul(
                        pp[:, 0:256],
                        xb8t[:, kc2, :, rt * 128:(rt + 1) * 128], wvs[kc2][:],
                        start=(kc2 == 0), stop=(kc2 == 3), perf_mode=DR)

                def ev(pp=pp, rt=rt):
                    nc.vector.tensor_copy(
                        vaug2[rt // 2][:, rt % 2, 4 * u:4 * u + 4, 0:D],
                        pp[:, 0:256].rearrange("p (h d) -> p h d", d=D))
                defer_ev(ev)
            return go
        return [mk(rt) for rt in range(T // 128)]

    # ---- K projection for one feature block f (4 rb-groups of 4 matmuls);
    # kth holds 64*(k + bk) in fp16 (bk comes host-scaled by 64)
    wks = {}

    def load_wk(f):
        wks[f] = pw.tile([128, 4, 2, 128], F8, tag="wqk", bufs=4,
                         name=f"wk{f}")
        nc.sync.dma_start(
            wks[f][:].rearrange("p a b c -> p (a b c)"),
            t_["wk_t"][f].rearrange("p a b c -> p (a b c)"))

    def emit_kproj_groups(f, kth_f):
        wk = wks.pop(f)

        def mk(rb):
            def go():
                pp = psP.tile([128, 512], F32, tag="proj")
                for kc2 in range(4):
                    nc.tensor.matmul(pp[:], wk[:, kc2, :, :],
                                     xb8[kc2][:, :, rb * 512:(rb + 1) * 512],
                                     start=(kc2 == 0), stop=(kc2 == 3),
                                     perf_mode=DR)
                defer_ev(lambda pp=pp, rb=rb: nc.vector.tensor_scalar_add(
                    kth_f[:, rb * 512:(rb + 1) * 512], pp[:],
                    ball[:, 8 + f:9 + f]))
            return go
        return [mk(rb) for rb in range(4)]

    # ---- emit: V(u0), K(0) up-front; then per-f attention with K(f+1)
    # and V(u1..u3) groups interleaved at unit boundaries.
    for thunk in emit_vproj_groups(0):
        thunk()
    load_wk(0)
    kth = [None, None]
    kth[0] = keep.tile([128, T], F16, tag="kth", bufs=2, name="kth0")
    for thunk in emit_kproj_groups(0, kth[0]):
        thunk()
    drain_ev()

    bg = []                # background PE work: (tag, thunk)
    bgx = []               # opportunistic MLP1 first-half partials
    pending = []           # AV matmuls deferred ~one block
    fin_q = []             # finalize chains deferred one (hp,qi) unit

    w1as = {}

    def load_w1a(m):
        w1as[m] = pw.tile([128, 4, 128], F16, tag="w1a", bufs=4,
                          name=f"w1a{m}")
        nc.sync.dma_start(
            w1as[m][:].rearrange("p k n -> p (k n)"),
            t_["w1a"][m].rearrange("p k n -> p (k n)"))

    def mk_partial(m):
        def go():
            if m + 2 < 32:
                load_w1a(m + 2)
            w1s = w1as.pop(m)
            pp = psP.tile([128, 2 * CH], F32, tag="proj")
            for kc in range(4):
                nc.tensor.matmul(pp[:], w1s[:, kc, :], x2r[kc][:],
                                 start=(kc == 0), stop=(kc == 3))
            defer_ev(lambda pp=pp, m=m: nc.vector.tensor_copy(
                p1[m][:], pp[:]))
        return go

    def flush():
        while len(pending) > 2:
            pending.pop(0)()
        while len(fin_q) > 1:
            fin_q.pop(0)()

    def bg_due(tag, f):
        kind, idx = tag
        return (kind == "k" and idx <= f) or (kind == "v" and f >= 2 * idx)

    for f in range(8):
        # schedule next-step projection work into this step's attention
        if f + 1 < 8:
            load_wk(f + 1)
            kth[(f + 1) % 2] = keep.tile([128, T], F16, tag="kth", bufs=2,
                                         name=f"kth{f + 1}")
            bg.extend((("k", f + 1), t)
                      for t in emit_kproj_groups(f + 1, kth[(f + 1) % 2]))
        if f in (0, 2, 4):
            u = f // 2 + 1
            bg.extend((("v", u), t) for t in emit_vproj_groups(u))
        if f == 4:
            # x2r[0..3] are final once every fin/cast up to f=3 is emitted
            # (the full drain below guarantees cast(3) is out); then MLP1's
            # first half can fill the ACT-bound attention tail
            while fin_q:
                fin_q.pop(0)()
            load_w1a(0)
            load_w1a(1)
            bgx.extend(mk_partial(m) for m in range(32))
        # anything attention(f) depends on MUST be emitted before it; pop
        # from the front until no due entries remain, then flush the last
        # deferred PSUM evacuation
        while any(bg_due(tag, f) for tag, _ in bg):
            bg.pop(0)[1]()
        drain_ev()
        kth_f = kth[f % 2]

        # Both heads of feature block f are processed together, chunk by
        # chunk: the 4 score matmuls of a block alternate PE row groups
        # (h0 at partitions 0:64, h1 at 64:128 — auto tile_position) so
        # they run pairwise-concurrent, and a single 1024-wide Exp covers
        # (h, s, q). Gate bias is per (slot, chunk) — identical for both
        # heads. Accumulators are per chunk with heads side by side in the
        # free dim, so the finalize (Ln/Exp/broadcast/normalize) also runs
        # once per chunk at width 512.
        unit_i = 0
        n_units = 12
        for (qi, qoff, nblk) in ((0, 0, 4), (1, CH, 8)):
            diag = nblk - 1
            acc = psA.tile([128, 2, CH], F32, tag="acc")
            for blk in range(nblk):
                ps = psS.tile([128, 2, 2, CH], F32, tag="ps")
                for s in range(2):
                    c = 2 * blk + s
                    for hp in range(2):
                        po = 64 * hp
                        nc.tensor.matmul(
                            ps[:, hp, s, :],
                            kth_f[po:po + D, c * 128:(c + 1) * 128],
                            qt[f][po:po + D, qoff:qoff + CH],
                            start=True, stop=True)
                ex = p2w.tile([128, 2, 2, CH], F8, tag="ex", bufs=6)
                if blk == diag:
                    sm = p2w.tile([128, 2, 2, CH], F32, tag="sm", bufs=2)
                    nc.vector.tensor_tensor(out=sm[:], in0=ps[:],
                                            in1=md[:], op=OP.add)
                    nc.scalar.activation(ex[:], sm[:], AF.Exp)
                else:
                    nc.scalar.activation(
                        ex[:], ps[:], AF.Exp,
                        bias=gt[:, 2 * blk + qi:2 * blk + qi + 1])
                flush()

                def mk_avs(ex=ex, blk=blk, f=f, acc=acc, nblk=nblk):
                    def go():
                        # start=True clears has_written for the WHOLE bank,
                        # not just the written region — so only the chunk's
                        # very first AV starts; h1's first write lands raw
                        # because the bank-wide clear covered its region too
                        for hp in range(2):
                            nc.tensor.matmul(
                                acc[0:D + 1, hp, :],
                                vaug2[blk][:, :, 2 * f + hp, :],
                                ex[:, hp, :, :],
                                start=(blk == 0 and hp == 0),
                                stop=(blk == nblk - 1), perf_mode=DR,
                                skip_group_check=True)
                    return go
                pending.append(mk_avs())

                # interleave background projection work between blocks,
                # paced so everything due by f+1 is emitted by end of f
                units_left = n_units - unit_i
                n_due = sum(1 for tag, _ in bg if bg_due(tag, f + 1))
                npop = max(1 if bg else 0, -(-n_due // units_left))
                for _ in range(npop):
                    if bg:
                        bg.pop(0)[1]()
                if bgx:
                    bgx.pop(0)()
                # drain chunk A's finalize mid-chunk-B so its accumulator
                # bank frees well before f+1's A chunk claims it
                if qi == 1 and blk == 3:
                    while fin_q:
                        fin_q.pop(0)()
                unit_i += 1

            while pending:
                pending.pop(0)()

            def mk_fin(acc=acc, f=f, qoff=qoff):
                def go():
                    # pb borrows the psP "proj" rotation below; flush any
                    # deferred projection evacuation first, else the slot
                    # parity shifts and a later chain clobbers a bank whose
                    # deferred evacuation isn't emitted yet
                    drain_ev()
                    lr = p2w.tile([128, 2, CH], F32, tag="lr", bufs=2)
                    nc.scalar.activation(lr[D:D + 1, :, :],
                                         acc[D:D + 1, :, :], AF.Ln)
                    rec = p2w.tile([128, 2, CH], F16, tag="rec", bufs=2)
                    nc.scalar.activation(rec[D:D + 1, :, :],
                                         lr[D:D + 1, :, :], AF.Exp,
                                         scale=-1.0)
                    # broadcast 1/den to 64 partitions (fp16 stream; the
                    # psum tile borrows the projection pool's rotation)
                    pb = psP.tile([128, 2, CH], F32, tag="proj")
                    nc.tensor.matmul(pb[0:D, :, :],
                                     ones16[D:D + 1, :],
                                     rec[D:D + 1, :, :],
                                     start=True, stop=True)
                    pbs = p2w.tile([128, 2, CH], F32, tag="pbs", bufs=2)
                    nc.vector.tensor_copy(pbs[0:D, :, :], pb[0:D, :, :])
                    tt = p2w.tile([128, 2, CH], F32, tag="tt", bufs=2)
                    nc.vector.tensor_tensor(
                        out=tt[0:D, :, :], in0=acc[0:D, :, :],
                        in1=pbs[0:D, :, :], op=OP.mult)
                    # head 0 -> partitions 0:64 of x2t, head 1 shifted to
                    # 64:128 with a gpsimd SBUF-SBUF DMA, adds aligned
                    nc.vector.tensor_tensor(
                        out=x2t[f][0:D, qoff:qoff + CH],
                        in0=tt[0:D, 0, :],
                        in1=xqt[f][0:D, qoff:qoff + CH], op=OP.add)
                    nc.gpsimd.dma_start(
                        x2t[f][D:128, qoff:qoff + CH], tt[0:D, 1, :])
                    nc.vector.tensor_tensor(
                        out=x2t[f][D:128, qoff:qoff + CH],
                        in0=x2t[f][D:128, qoff:qoff + CH],
                        in1=xqt[f][D:128, qoff:qoff + CH], op=OP.add)
                return go
            fin_q.append(mk_fin())
        # end chunk loop: cast x2t[f] -> fp16 once both chunks finalized
        def mk_cast(f=f):
            def go():
                nc.vector.tensor_copy(x2r[f][:], x2t[f][:])
            return go
        fin_q.append(mk_cast())

    while bg:
        bg.pop(0)[1]()
    while bgx:
        bgx.pop(0)()
    while pending:
        pending.pop(0)()
    # prefetch the first phase-3 w1b tiles on the now-idle sync queue
    for m in range(4):
        pre3_w1[m] = pre3.tile([128, 4, 128], F16, name=f"p3w1_{m}")
        nc.sync.dma_start(
            pre3_w1[m][:].rearrange("p k n -> p (k n)"),
            t_["w1b"][m].rearrange("p k n -> p (k n)"))
    for fn in fin_q:
        fn()
    fin_q.clear()
    drain_ev()


def _phase3(nc, tc, AF, OP, t_, ball, x2t, x2r, p1, pre3_w1):
    """MLP (fp16, transposed) + residual; output stays transposed. MLP1's
    kc 0..3 half was precomputed into p1 during the attention tail; here we
    do kc 4..7 and recombine before the Relu."""
    w1_t, w2_t, out_t = t_["w1b"], t_["w2_t"], t_["out_t"]
    with tc.tile_pool(name="p3h", bufs=1) as p3h, \
         tc.tile_pool(name="p3w", bufs=2) as p3w, \
         tc.tile_pool(name="p3y", bufs=2) as p3y, \
         tc.tile_pool(name="ps3", bufs=6, space="PSUM") as ps3:

        ht = [p3h.tile([128, 4, 2 * CH], F16, tag=f"ht{i}", name=f"ht{i}")
              for i in range(8)]
        w1ss = dict(pre3_w1)

        def load_w1(m):
            w1ss[m] = p3w.tile([128, 4, 128], F16, tag="w1s", bufs=3,
                               name=f"w1s{m}")
            nc.sync.dma_start(
                w1ss[m][:].rearrange("p k n -> p (k n)"),
                w1_t[m].rearrange("p k n -> p (k n)"))

        ev_pend = [None]
        for m in range(32):
            if 4 <= m + 4 < 32:
                load_w1(m + 4)
            w1s = w1ss.pop(m)
            pp = ps3.tile([128, 2 * CH], F32, tag="proj")
            for kc in range(4):
                nc.tensor.matmul(pp[:], w1s[:, kc, :], x2r[4 + kc][:],
                                 start=(kc == 0), stop=(kc == 3))
            if ev_pend[0] is not None:
                ev_pend[0]()

            def ev(pp=pp, m=m):
                ts = p3y.tile([128, 2 * CH], F32, tag="ts", bufs=2,
                              name=f"ts{m}")
                nc.vector.tensor_tensor(out=ts[:], in0=pp[:], in1=p1[m][:],
                                        op=OP.add)
                nc.scalar.activation(ht[m // 4][:, m % 4, :], ts[:], AF.Relu,
                                     bias=bias["b1"][:, m:m + 1])
            ev_pend[0] = ev
        ev_pend[0]()
        ev_pend[0] = None

        w2ss = {}
        w2fs = {}

        def load_w2(mo):
            w2ss[mo] = p3w.tile([128, 32, 128], F16, tag="w2s", bufs=2,
                                name=f"w2s{mo}")
            nc.sync.dma_start(
                w2ss[mo][:].rearrange("p k n -> p (k n)"),
                w2_t[mo].rearrange("p k n -> p (k n)"))

        load_w2(0)
        load_w2(1)
        for mo in range(8):
            if mo + 2 < 8:
                load_w2(mo + 2)
            w2s = w2ss.pop(mo)
            pp = ps3.tile([128, 2 * CH], F32, tag="proj")
            for kc in range(32):
                nc.tensor.matmul(pp[:], w2s[:, kc, :],
                                 ht[kc // 4][:, kc % 4, :],
                                 start=(kc == 0), stop=(kc == 31))
            ys = p3y.tile([128, 2 * CH], F32, tag="ys", bufs=2, name=f"ys{mo}")
            nc.scalar.activation(ys[:], pp[:], AF.Identity,
                                 bias=ball[:, 56 + mo:57 + mo])
            nc.vector.tensor_tensor(out=ys[:], in0=ys[:], in1=x2t[mo][:],
                                    op=OP.add)
            eng = (nc.gpsimd, nc.sync, nc.scalar)[mo % 3]
            eng.dma_start(out_t[mo * 128:(mo + 1) * 128, :], ys[:])


# --------------------------------------------------------------- host driver
def _install_ntff_hook():
    """The container's antenv stub lacks axon_hooks; provide it so
    run_bass_kernel_spmd(trace=True) can capture NTFF profiles via libaxon."""
    import types

    try:
        import antenv.axon_hooks  # noqa: F401
        return
    except ImportError:
        pass
    holder = {"h": None}
    mod = types.ModuleType("antenv.axon_hooks")
    mod.set_axon_ntff_profile_hook = lambda h: holder.__setitem__("h", h)
    mod.get_axon_ntff_profile_hook = lambda: holder["h"]
    sys.modules["antenv.axon_hooks"] = mod
    import antenv

    antenv.axon_hooks = mod
    if "/root/.axon_site" not in sys.path:
        sys.path.insert(0, "/root/.axon_site")
    from trn_agent_boot.trn_boot import _ntff_profile_via_ctypes

    so = "/opt/axon/libaxon_pjrt.so"
    if os.path.exists(so):
        mod.set_axon_ntff_profile_hook(_ntff_profile_via_ctypes(so))


def _get_program():
    key = ("v16",)
    if key not in _prog_cache:
        nc = _build_program()
        _legalize_waits(nc)
        _prog_cache[key] = nc
    return _prog_cache[key]


def _prep_shared(Wqkv, W1, W2, bqkv, b1, b2):
    import ml_dtypes
    f8 = ml_dtypes.float8_e4m3
    Wq, Wk, Wv = Wqkv[:, :N], Wqkv[:, N:2 * N], Wqkv[:, 2 * N:]
    # [f][p][kc2][i][n] = 64*W[(2*kc2+i)*128+p, f*128+n]
    wq_t = np.ascontiguousarray(
        (Wq * W8SCALE).reshape(4, 2, 128, 8, 128)
        .transpose(2, 3, 0, 1, 4).astype(f8))
    wk_t = np.ascontiguousarray(
        (Wk * W8SCALE).reshape(4, 2, 128, 8, 128)
        .transpose(2, 3, 0, 1, 4).astype(f8))
    # [u][kc2][p][i][n] = 64*Wv[(2*kc2+i)*128+p, u*256+n]
    wv_t = np.ascontiguousarray(
        (Wv * W8SCALE).reshape(4, 2, 128, 4, 256)
        .transpose(3, 0, 2, 1, 4).astype(f8))
    w1f = W1.reshape(8, 128, 32, 128).transpose(2, 1, 0, 3) \
        .astype(np.float16)
    w1a = np.ascontiguousarray(w1f[:, :, 0:4, :])
    w1b = np.ascontiguousarray(w1f[:, :, 4:8, :])
    w2_t = np.ascontiguousarray(
        W2.reshape(32, 128, 8, 128).transpose(2, 1, 0, 3)
        .astype(np.float16))

    def pf(v):
        # bias layout [128, w]: element [p, f] = v[f*128 + p]
        return np.ascontiguousarray(v.reshape(-1, 128).T)

    return {
        "wq_t": wq_t, "wk_t": wk_t, "wv_t": wv_t,
        "w1a": w1a, "w1b": w1b, "w2_t": w2_t, "w2_f": w2_f,
        "bqs": pf(bqkv[:N] * (0.125 / W8SCALE)),
        "bk": pf(bqkv[N:2 * N] * W8SCALE),
        "bv": pf(bqkv[2 * N:]),
        "b1": pf(b1), "b2": pf(b2),
    }


def _core_chunks(c):
    b, j = c // 4, c % 4
    return b, j, 7 - j


def _slot_blocks(j):
    # slot order of the 8 kv row-blocks: slot 3 = A diag (block j),
    # slot 7 = B diag (block 7-j), others ascending.
    other = [b for b in range(8) if b not in (j, 7 - j)]
    return [other[0], other[1], other[2], j, other[3], other[4], other[5],
            7 - j]


def _make_gates(j):
    slots = _slot_blocks(j)
    g = np.full((128, 16), -1e9, np.float32)
    for s in range(8):
        if s != 3 and slots[s] < j:
            g[:, 2 * s] = 0.0          # allowed for A
        if s != 7 and slots[s] < 7 - j:
            g[:, 2 * s + 1] = 0.0      # allowed for B
    return g


_MASKD = np.where(np.arange(256)[:, None] <= np.arange(CH)[None, :],
                  0.0, -1e9).astype(np.float32)


def kernel(x, Wqkv, bqkv, W1, b1, W2, b2, _trace=False):
    x = np.asarray(x, dtype=np.float32)
    shared = _prep_shared(np.asarray(Wqkv, np.float32),
                          np.asarray(W1, np.float32),
                          np.asarray(W2, np.float32),
                          np.asarray(bqkv, np.float32),
                          np.asarray(b1, np.float32),
                          np.asarray(b2, np.float32))
    in_maps = []
    for c in range(NCORES):
        b, j, jb = _core_chunks(c)
        xqc = np.concatenate(
            [x[b, j * CH:(j + 1) * CH], x[b, jb * CH:(jb + 1) * CH]], axis=0)
        import ml_dtypes
        f8 = ml_dtypes.float8_e4m3
        xqt = np.ascontiguousarray(xqc.T)
        xbp = x[b].reshape(8, CH, N)[_slot_blocks(j)].reshape(T, N)
        in_maps.append({
            **shared,
            "xqt": xqt,
            "xq8": np.ascontiguousarray(
                xqt.reshape(4, 2, 128, 2 * CH).transpose(0, 2, 1, 3)
                .astype(f8)),
            "xb8": np.ascontiguousarray(
                xbp.T.reshape(4, 2, 128, T).transpose(2, 0, 1, 3)
                .astype(f8)),
            "gates": _make_gates(j), "maskd": _MASKD,
        })

    nc = _get_program()
    if _trace:
        _install_ntff_hook()
    res = run_bass_kernel_spmd(nc, in_maps, list(range(NCORES)), trace=_trace)

    outf = np.empty((B, T, N), dtype=np.float32)
    for c in range(NCORES):
        b, j, jb = _core_chunks(c)
        o = np.ascontiguousarray(res.results[c]["out_t"].T)
        outf[b, j * CH:(j + 1) * CH] = o[:CH]
        outf[b, jb * CH:(jb + 1) * CH] = o[CH:]
    if _trace:
        kernel.last_results = res
    return outf

